# revision 18
# baseline (speedup 1.0000x reference)
"""PathCausalSelfAttention on 8 trn2 cores.

Sharding: core c -> batch b=c//4, head-group hg=c%4 (4 heads each).

Key simplification vs the reference: the x-path score term is weighted
1e-6 and contributes ~1e-6 relative to the g-path, far below the 2e-2
tolerance, so q/k projections are dropped entirely. Scores are g.g per
head, exp on ACT, PV with a fused ones column for the softmax
denominator, out-projection in bf16.

v2 structure: ScalarE (exp over the causal triangle, ~57us of columns
at 0.82ns/col) pays ~293ns fixed cost per ACTIVATE, so score tiles are
built in two wide PSUM regions (A: 4 banks/2048 cols for key tiles
j<8, B: 2 banks/1024 cols for j>=8, emitted interleaved j,j+8 so the
regions ping-pong against the single act per (h,j) -- 64 acts total vs
96 before). Normalization: reciprocal on DVE straight from the PV
denominator row, gpsimd partition_broadcast of the fp32 row, then one
fused scalar_tensor_tensor multiply-evict into ytsb. The out-projection
rides the last head's quarters, borrowing the then-idle A/B PSUM banks.
Causal diag masks on GpSimd (SBUF only). Host sums 4 bf16 head-group
partials per batch.
"""

import numpy as np
import ml_dtypes

import concourse.bacc as bacc
import concourse.mybir as mybir
import concourse.tile as tile
from concourse import masks
from concourse.bass_utils import run_bass_kernel_spmd

B, L, D, H = 2, 2048, 1024, 16
HD = 64
NCORES = 8
NH = 4          # heads per core
PC = NH * HD    # 256 v / out-proj rows per core
FP = mybir.dt.float32
BF = mybir.dt.bfloat16
F8 = mybir.dt.float8e4
DR = mybir.MatmulPerfMode.DoubleRow
AF = mybir.ActivationFunctionType
ALU = mybir.AluOpType

LT = L // 128   # 16 L-tiles
DC = D // 128   # 8 contraction chunks
VW = 256  # V' cols per L-tile per pr: 2 heads x [ones | 63 pad | 64 v dims]

# j emission order: big tile (region A) alternating with small (region B)
JORDER = [0, 8, 1, 9, 2, 10, 3, 11, 4, 12, 5, 13, 6, 14, 7, 15]

ACT_W = 1024    # max exp width per ACTIVATE

DEBUG = False


def _emit(nc, tc):
    xT_d = nc.declare_dram_parameter("xT", [D, L], BF, isOutput=False)
    # per-head zero-padded key tiles: head h's 64 dims live in partition
    # rows 64*(h%2)..+64, other rows zero. Used as the score lhsT so the
    # contraction is full 128 rows and lhsT/rhs come from different tiles.
    gT_d = nc.declare_dram_parameter("gT", [PC, L], BF, isOutput=False)
    gz_d = nc.declare_dram_parameter("gz", [NH * 128, L], BF, isOutput=False)
    wv_d = nc.declare_dram_parameter("wv", [D, PC], BF, isOutput=False)
    wo_d = nc.declare_dram_parameter("wo", [PC, D], BF, isOutput=False)
    out_p = nc.declare_dram_parameter("out_p", [L, D], BF, isOutput=True)

    perm = tc.alloc_tile_pool(name="perm", bufs=1)
    I128 = perm.tile([128, 128], BF, name="I128")
    An = perm.tile([128, 128], BF, name="An")
    gt = [perm.tile([128, L], BF, name=f"gt{p}") for p in range(2)]
    gz = [perm.tile([128, L], BF, name=f"gz{h}") for h in range(NH)]
    xT = [perm.tile([128, L], BF, name=f"xT{d}") for d in range(DC)]
    wv = [perm.tile([128, PC], BF, name=f"wv{d}") for d in range(DC)]
    wo = [perm.tile([128, D], BF, name=f"wo{p}") for p in range(2)]
    vp = [perm.tile([128, LT * VW], BF, name=f"vp{p}") for p in range(2)]
    ytsb = [perm.tile([128, L], BF, name=f"yt{p}") for p in range(2)]
    # two ping-pong sets of per-head p tiles, trimmed to the causal width
    pts = [[perm.tile([128, L - 128 * j], BF, name=f"pt{s}_{j}")
            for j in range(LT)] for s in range(2)]
    rcp = [perm.tile([HD, 512], FP, name=f"rcp{i}") for i in range(2)]
    perm.seal()

    # DMAs: first-needed first. gz0/gt0 column-chunked so the first score
    # matmuls start as soon as the leading columns land.
    # DMAs: first-needed first. gz0/gt0 column-chunked so the first score
    # matmuls start as soon as the leading columns land.
    nc.sync.dma_start(out=gz[0][:, 0:128], in_=gz_d[0:128, 0:128])
    bounds = [0, 512, 1024, 1536, 2048]
    for c4 in range(len(bounds) - 1):
        sl = slice(bounds[c4], bounds[c4 + 1])
        nc.sync.dma_start(out=gt[0][:, sl], in_=gT_d[0:128, sl])
    nc.sync.dma_start(out=gz[0][:, 128:1024], in_=gz_d[0:128, 128:1024])
    nc.sync.dma_start(out=gz[0][:, 1024:2048], in_=gz_d[0:128, 1024:2048])
    for d in range(DC):
        nc.sync.dma_start(out=xT[d], in_=xT_d[128 * d:128 * (d + 1), :])
    for d in range(DC):
        nc.sync.dma_start(out=wv[d], in_=wv_d[128 * d:128 * (d + 1), :])
    nc.sync.dma_start(out=gz[1], in_=gz_d[128:256, :])
    nc.sync.dma_start(out=gt[1], in_=gT_d[128:256, :])
    nc.sync.dma_start(out=gz[2], in_=gz_d[256:384, :])
    nc.sync.dma_start(out=gz[3], in_=gz_d[384:512, :])
    for p in range(2):
        nc.sync.dma_start(out=wo[p], in_=wo_d[128 * p:128 * (p + 1), :])

    masks.make_identity(nc, I128)
    masks.make_lower_triangular(nc, An, val=-240.0, diag=False)
    for p in range(2):
        # each 128-col head group is [64 ones | 64 v dims]: the PV matmul
        # then lands the softmax denominator on yq partitions 0..63
        # (replicated) and y on 64..127, so 1/den runs on DVE with no
        # cross-partition broadcast at all
        nc.vector.memset(vp[p], 1.0)

    def score_tile(scA, scB, h, s, j):
        """Score chain for (h, key tile j): matmuls into one wide PSUM
        region with the causal mask of the diagonal block folded in as an
        additive -240 matmul (I128.T @ An), then ONE exp act."""
        pr = h // 2
        W = L - 128 * j
        pool, cap = (scA, 2048) if j < 8 else (scB, 1024)
        ptj = pts[s][j]
        rt = pool.tile([128, cap], FP, name="rt")
        nc.tensor.matmul(rt[:, 0:128], lhsT=I128, rhs=An,
                         start=True, stop=False)
        nc.tensor.matmul(
            rt[:, 0:128],
            lhsT=gz[h][:, 128 * j:128 * j + 128],
            rhs=gt[pr][:, 128 * j:128 * j + 128],
            start=False, stop=True)
        c = 128
        while c < W:
            bw = min(512 - (c & 511), W - c)
            q0 = 128 * j + c
            nc.tensor.matmul(
                rt[:, c:c + bw],
                lhsT=gz[h][:, 128 * j:128 * j + 128],
                rhs=gt[pr][:, q0:q0 + bw],
                start=True, stop=True)
            c += bw
        nc.scalar.activation(ptj[:, 0:W], rt[:, 0:W], AF.Exp, scale=0.125)

    def vproj_tile(vpool, i):
        ps = vpool.tile([128, PC], FP, name="vps", padded_shape=[128, 512])
        for d in range(DC):
            nc.tensor.matmul(
                ps, lhsT=xT[d][:, 128 * i:128 * (i + 1)],
                rhs=wv[d],
                start=(d == 0), stop=(d == DC - 1))
        for pr in range(2):
            # both heads' 64-col v blocks in one strided cast per pr; cols
            # 0..63 of each group stay 1.0 (replicated softmax denominator)
            nc.vector.tensor_copy(
                vp[pr][:, VW * i:VW * i + VW].rearrange(
                    "p (hh c) -> p hh c", hh=2)[:, :, 64:64 + HD],
                ps[:, 128 * pr:128 * (pr + 1)].rearrange(
                    "p (hh c) -> p hh c", hh=2))

    def pv_quarter_mm(yq, h, s, qt, jj):
        pr, hh = h // 2, h % 2
        q0 = max(128 * jj, 512 * qt)
        q1 = 512 * (qt + 1)
        nc.tensor.matmul(
            yq[:, q0 - 512 * qt:q1 - 512 * qt],
            lhsT=vp[pr][:, VW * jj + 128 * hh:VW * jj + 128 * hh + 128],
            rhs=pts[s][jj][:, q0 - 128 * jj:q1 - 128 * jj],
            start=(jj == 0), stop=(jj == 4 * qt + 3))

    def quarter_tail(yq, h, qt):
        """After PV quarter qt stops: 1/den on DVE over the 64 replicated
        den rows, then one fused multiply-evict into ytsb."""
        pr, hh = h // 2, h % 2
        o = 512 * qt
        i = (4 * h + qt) % 2
        nc.vector.reciprocal_approx_fast(rcp[i][0:HD, 0:512],
                                         yq[0:HD, :])
        nc.vector.scalar_tensor_tensor(
            out=ytsb[pr][64 * hh:64 * hh + 64, o:o + 512],
            in0=yq[64:128, :], scalar=0.0, in1=rcp[i][0:HD, 0:512],
            op0=ALU.bypass, op1=ALU.mult)

    def outproj_chunk(scA, scB, obpool, lt, n2):
        # reuse the score-region slots (same tag) -- the A/B banks are idle
        # during the last head's quarters
        if (lt + n2) % 2 == 0:
            op = scA.tile([128, 512], FP, name="rt",
                          padded_shape=[128, 2048])
        else:
            op = scB.tile([128, 512], FP, name="rt",
                          padded_shape=[128, 1024])
        for pr in range(2):
            nc.tensor.matmul(
                op, lhsT=ytsb[pr][:, 128 * lt:128 * (lt + 1)],
                rhs=wo[pr][:, 512 * n2:512 * (n2 + 1)],
                start=(pr == 0), stop=(pr == 1))
        ob = obpool.tile([128, 512], BF, name="ob")
        # alternate evict engines: ScalarE is idle during the last head
        if (lt + n2) % 2 == 0:
            nc.vector.tensor_copy(ob, op)
        else:
            nc.scalar.copy(ob, op)
        nc.sync.dma_start(
            out=out_p[128 * lt:128 * (lt + 1), 512 * n2:512 * (n2 + 1)],
            in_=ob)

    with (
        tc.tile_pool(name="scA", bufs=1, space="PSUM") as scA,
        tc.tile_pool(name="scB", bufs=1, space="PSUM") as scB,
    ):
        # phase A/B: scores for head 0 with the v-projection interleaved
        with tc.tile_pool(name="vps", bufs=2, space="PSUM") as vpool:
            for idx, j in enumerate(JORDER):
                score_tile(scA, scB, 0, 0, j)
                if idx >= 6:
                    vproj_tile(vpool, idx - 6)
            for i in range(LT - 6, LT):
                vproj_tile(vpool, i)

        # slots: PV(h) quarter-major, scores of head h+1 spread through;
        # the out-projection rides slot 3 per normalized query quarter.
        with (
            tc.tile_pool(name="yT", bufs=2, space="PSUM") as ypool,
            tc.tile_pool(name="ob", bufs=4) as obpool,
        ):
            for h in range(NH):
                s = h % 2
                emitted = 0
                k = 0
                for qt in range(4):
                    yq = ypool.tile([128, 512], FP, name="yT")
                    for jj in range(4 * qt + 4):
                        if h + 1 < NH and emitted < LT and emitted <= k * 16 // 40:
                            score_tile(scA, scB, h + 1, (h + 1) % 2,
                                       JORDER[emitted])
                            emitted += 1
                        pv_quarter_mm(yq, h, s, qt, jj)
                        k += 1
                    quarter_tail(yq, h, qt)
                    if h == NH - 1:
                        for lt in range(4 * qt, 4 * qt + 4):
                            for n2 in range(2):
                                outproj_chunk(scA, scB, obpool, lt, n2)
                while h + 1 < NH and emitted < LT:
                    score_tile(scA, scB, h + 1, (h + 1) % 2, JORDER[emitted])
                    emitted += 1

    if DEBUG:
        dbg_yt = nc.declare_dram_parameter("dbg_yt", [256, L], BF,
                                           isOutput=True)
        dbg_vp = nc.declare_dram_parameter("dbg_vp", [256, LT * VW], BF,
                                           isOutput=True)
        dbg_pt = nc.declare_dram_parameter("dbg_pt", [128, L], BF,
                                           isOutput=True)
        for p in range(2):
            nc.sync.dma_start(out=dbg_yt[128 * p:128 * (p + 1), :],
                              in_=ytsb[p])
            nc.sync.dma_start(out=dbg_vp[128 * p:128 * (p + 1), :],
                              in_=vp[p])
        nc.sync.dma_start(out=dbg_pt[:, 0:L], in_=pts[0][0])
    perm.release()


_NC = None


def build_nc():
    global _NC
    if _NC is None:
        nc = bacc.Bacc("TRN2", target_bir_lowering=False)
        with tile.TileContext(nc) as tc:
            _emit(nc, tc)
        nc.finalize()
        _NC = nc
    return _NC


def prep_in_maps(x, g, W_qkv, W_out):
    x = np.asarray(x, dtype=np.float32)
    g = np.asarray(g, dtype=np.float32)
    W_qkv = np.asarray(W_qkv, dtype=np.float32)
    W_out = np.asarray(W_out, dtype=np.float32)
    bf = ml_dtypes.bfloat16
    xT = [np.ascontiguousarray(x[b].T).astype(bf) for b in range(B)]
    in_maps = []
    for c in range(NCORES):
        b, hg = c // 4, c % 4
        lo = PC * hg
        gTb = np.ascontiguousarray(g[b][:, lo:lo + PC].T).astype(bf)
        gzb = np.zeros((NH * 128, L), dtype=bf)
        for h in range(NH):
            r = 64 * (h % 2)
            gzb[128 * h + r:128 * h + r + 64, :] = gTb[64 * h:64 * h + 64, :]
        in_maps.append({
            "xT": xT[b],
            "gT": gTb,
            "gz": gzb,
            "wv": np.ascontiguousarray(
                W_qkv[:, 2 * D + lo:2 * D + lo + PC]).astype(bf),
            "wo": np.ascontiguousarray(W_out[lo:lo + PC, :]).astype(bf),
        })
    return in_maps


def gather(results):
    out = np.zeros((B, L, D), dtype=np.float32)
    for c in range(NCORES):
        out[c // 4] += np.asarray(results[c]["out_p"], dtype=np.float32)
    return out


def kernel(x, g, W_qkv, W_out):
    nc = build_nc()
    in_maps = prep_in_maps(x, g, W_qkv, W_out)
    res = run_bass_kernel_spmd(nc, in_maps, list(range(NCORES)))
    return gather(res.results)


# revision 19
# speedup vs baseline: 1.0910x; 1.0910x over previous
"""PathCausalSelfAttention on 8 trn2 cores.

Sharding: core c -> batch b=c//4, head-group hg=c%4 (4 heads each).

Key simplification vs the reference: the x-path score term is weighted
1e-6 and contributes ~1e-6 relative to the g-path, far below the 2e-2
tolerance, so q/k projections are dropped entirely. Scores are g.g per
head, exp on ACT, PV with a fused ones column for the softmax
denominator, out-projection in bf16.

v2 structure: ScalarE (exp over the causal triangle, ~57us of columns
at 0.82ns/col) pays ~293ns fixed cost per ACTIVATE, so score tiles are
built in two wide PSUM regions (A: 4 banks/2048 cols for key tiles
j<8, B: 2 banks/1024 cols for j>=8, emitted interleaved j,j+8 so the
regions ping-pong against the single act per (h,j) -- 64 acts total vs
96 before). Normalization: reciprocal on DVE straight from the PV
denominator row, gpsimd partition_broadcast of the fp32 row, then one
fused scalar_tensor_tensor multiply-evict into ytsb. The out-projection
rides the last head's quarters, borrowing the then-idle A/B PSUM banks.
Causal diag masks on GpSimd (SBUF only). Host sums 4 bf16 head-group
partials per batch.
"""

import numpy as np
import ml_dtypes

import concourse.bacc as bacc
import concourse.mybir as mybir
import concourse.tile as tile
from concourse import masks
from concourse.bass_utils import run_bass_kernel_spmd

B, L, D, H = 2, 2048, 1024, 16
HD = 64
NCORES = 8
NH = 4          # heads per core
PC = NH * HD    # 256 v / out-proj rows per core
FP = mybir.dt.float32
BF = mybir.dt.bfloat16
F8 = mybir.dt.float8e4
DR = mybir.MatmulPerfMode.DoubleRow
AF = mybir.ActivationFunctionType
ALU = mybir.AluOpType

LT = L // 128   # 16 L-tiles
DC = D // 128   # 8 contraction chunks
VW = 256  # V' cols per L-tile per pr: 2 heads x [ones | 63 pad | 64 v dims]

# j emission order: big tile (region A) alternating with small (region B)
JORDER = [0, 8, 1, 9, 2, 10, 3, 11, 4, 12, 5, 13, 6, 14, 7, 15]

ACT_W = 1024    # max exp width per ACTIVATE

DEBUG = False


def _emit(nc, tc):
    xT_d = nc.declare_dram_parameter("xT", [D, L], BF, isOutput=False)
    # per-head zero-padded key tiles: head h's 64 dims live in partition
    # rows 64*(h%2)..+64, other rows zero. Used as the score lhsT so the
    # contraction is full 128 rows and lhsT/rhs come from different tiles.
    gT_d = nc.declare_dram_parameter("gT", [PC, L], BF, isOutput=False)
    gz_d = nc.declare_dram_parameter("gz", [NH * 128, L], BF, isOutput=False)
    wv_d = nc.declare_dram_parameter("wv", [D, PC], BF, isOutput=False)
    wo_d = nc.declare_dram_parameter("wo", [PC, D], BF, isOutput=False)
    out_p = nc.declare_dram_parameter("out_p", [L, D], BF, isOutput=True)

    perm = tc.alloc_tile_pool(name="perm", bufs=1)
    I128 = perm.tile([128, 128], BF, name="I128")
    An = perm.tile([128, 128], BF, name="An")
    gt = [perm.tile([128, L], BF, name=f"gt{p}") for p in range(2)]
    gz = [perm.tile([128, L], BF, name=f"gz{h}") for h in range(NH)]
    xT = [perm.tile([128, L], BF, name=f"xT{d}") for d in range(DC)]
    wv = [perm.tile([128, PC], BF, name=f"wv{d}") for d in range(DC)]
    wo = [perm.tile([128, D], BF, name=f"wo{p}") for p in range(2)]
    vp = [perm.tile([128, LT * VW], BF, name=f"vp{p}") for p in range(2)]
    ytsb = [perm.tile([128, L], BF, name=f"yt{p}") for p in range(2)]
    # two ping-pong sets of per-head p tiles, trimmed to the causal width
    pts = [[perm.tile([128, L - 128 * j], BF, name=f"pt{s}_{j}")
            for j in range(LT)] for s in range(2)]
    rcp = [perm.tile([HD, 512], FP, name=f"rcp{i}") for i in range(2)]
    perm.seal()

    # DMAs: first-needed first. gz0/gt0 column-chunked so the first score
    # matmuls start as soon as the leading columns land.
    # DMAs: first-needed first. gz0/gt0 column-chunked so the first score
    # matmuls start as soon as the leading columns land.
    bounds = [0, 256, 512, 1024, 1536, 2048]
    for c4 in range(len(bounds) - 1):
        sl = slice(bounds[c4], bounds[c4 + 1])
        nc.sync.dma_start(out=gz[0][:, sl], in_=gz_d[0:128, sl])
        nc.sync.dma_start(out=gt[0][:, sl], in_=gT_d[0:128, sl])
    for d in range(DC):
        nc.sync.dma_start(out=xT[d], in_=xT_d[128 * d:128 * (d + 1), :])
    for d in range(DC):
        nc.sync.dma_start(out=wv[d], in_=wv_d[128 * d:128 * (d + 1), :])
    nc.sync.dma_start(out=gz[1], in_=gz_d[128:256, :])
    nc.sync.dma_start(out=gt[1], in_=gT_d[128:256, :])
    nc.sync.dma_start(out=gz[2], in_=gz_d[256:384, :])
    nc.sync.dma_start(out=gz[3], in_=gz_d[384:512, :])
    for p in range(2):
        nc.sync.dma_start(out=wo[p], in_=wo_d[128 * p:128 * (p + 1), :])

    masks.make_identity(nc, I128)
    masks.make_lower_triangular(nc, An, val=-240.0, diag=False)
    for p in range(2):
        # each 128-col head group is [64 ones | 64 v dims]: the PV matmul
        # then lands the softmax denominator on yq partitions 0..63
        # (replicated) and y on 64..127, so 1/den runs on DVE with no
        # cross-partition broadcast at all
        nc.vector.memset(vp[p], 1.0)

    def score_tile(scA, scB, h, s, j):
        """Score chain for (h, key tile j): matmuls into one wide PSUM
        region with the causal mask of the diagonal block folded in as an
        additive -240 matmul (I128.T @ An), then ONE exp act."""
        pr = h // 2
        W = L - 128 * j
        pool, cap = (scA, 2048) if j < 8 else (scB, 1024)
        ptj = pts[s][j]
        rt = pool.tile([128, cap], FP, name="rt")
        nc.tensor.matmul(rt[:, 0:128], lhsT=I128, rhs=An,
                         start=True, stop=False)
        nc.tensor.matmul(
            rt[:, 0:128],
            lhsT=gz[h][:, 128 * j:128 * j + 128],
            rhs=gt[pr][:, 128 * j:128 * j + 128],
            start=False, stop=True)
        c = 128
        while c < W:
            bw = min(512 - (c & 511), W - c)
            q0 = 128 * j + c
            nc.tensor.matmul(
                rt[:, c:c + bw],
                lhsT=gz[h][:, 128 * j:128 * j + 128],
                rhs=gt[pr][:, q0:q0 + bw],
                start=True, stop=True)
            c += bw
        nc.scalar.activation(ptj[:, 0:W], rt[:, 0:W], AF.Exp, scale=0.125)

    def vproj_tile(vpool, i):
        ps = vpool.tile([128, PC], FP, name="vps", padded_shape=[128, 512])
        for d in range(DC):
            nc.tensor.matmul(
                ps, lhsT=xT[d][:, 128 * i:128 * (i + 1)],
                rhs=wv[d],
                start=(d == 0), stop=(d == DC - 1))
        for pr in range(2):
            # both heads' 64-col v blocks in one strided cast per pr; cols
            # 0..63 of each group stay 1.0 (replicated softmax denominator)
            nc.vector.tensor_copy(
                vp[pr][:, VW * i:VW * i + VW].rearrange(
                    "p (hh c) -> p hh c", hh=2)[:, :, 64:64 + HD],
                ps[:, 128 * pr:128 * (pr + 1)].rearrange(
                    "p (hh c) -> p hh c", hh=2))

    def pv_quarter_mm(yq, h, s, qt, jj):
        pr, hh = h // 2, h % 2
        q0 = max(128 * jj, 512 * qt)
        q1 = 512 * (qt + 1)
        nc.tensor.matmul(
            yq[:, q0 - 512 * qt:q1 - 512 * qt],
            lhsT=vp[pr][:, VW * jj + 128 * hh:VW * jj + 128 * hh + 128],
            rhs=pts[s][jj][:, q0 - 128 * jj:q1 - 128 * jj],
            start=(jj == 0), stop=(jj == 4 * qt + 3))

    def quarter_tail(yq, h, qt):
        """After PV quarter qt stops: 1/den on DVE over the 64 replicated
        den rows, then one fused multiply-evict into ytsb."""
        pr, hh = h // 2, h % 2
        o = 512 * qt
        i = (4 * h + qt) % 2
        nc.vector.reciprocal_approx_fast(rcp[i][0:HD, 0:512],
                                         yq[0:HD, :])
        nc.vector.scalar_tensor_tensor(
            out=ytsb[pr][64 * hh:64 * hh + 64, o:o + 512],
            in0=yq[64:128, :], scalar=0.0, in1=rcp[i][0:HD, 0:512],
            op0=ALU.bypass, op1=ALU.mult)

    def outproj_chunk(scA, scB, obpool, lt, n2):
        # reuse the score-region slots (same tag) -- the A/B banks are idle
        # during the last head's quarters
        if (lt + n2) % 2 == 0:
            op = scA.tile([128, 512], FP, name="rt",
                          padded_shape=[128, 2048])
        else:
            op = scB.tile([128, 512], FP, name="rt",
                          padded_shape=[128, 1024])
        for pr in range(2):
            nc.tensor.matmul(
                op, lhsT=ytsb[pr][:, 128 * lt:128 * (lt + 1)],
                rhs=wo[pr][:, 512 * n2:512 * (n2 + 1)],
                start=(pr == 0), stop=(pr == 1))
        ob = obpool.tile([128, 512], BF, name="ob")
        # alternate evict engines: ScalarE is idle during the last head
        if (lt + n2) % 2 == 0:
            nc.vector.tensor_copy(ob, op)
        else:
            nc.scalar.copy(ob, op)
        nc.sync.dma_start(
            out=out_p[128 * lt:128 * (lt + 1), 512 * n2:512 * (n2 + 1)],
            in_=ob)

    with (
        tc.tile_pool(name="scA", bufs=1, space="PSUM") as scA,
        tc.tile_pool(name="scB", bufs=1, space="PSUM") as scB,
    ):
        # phase A/B: scores for head 0 with the v-projection interleaved
        with tc.tile_pool(name="vps", bufs=2, space="PSUM") as vpool:
            for idx, j in enumerate(JORDER):
                score_tile(scA, scB, 0, 0, j)
                if idx >= 6:
                    vproj_tile(vpool, idx - 6)
            for i in range(LT - 6, LT):
                vproj_tile(vpool, i)

        # slots: PV(h) quarter-major, scores of head h+1 spread through;
        # the out-projection rides slot 3 per normalized query quarter.
        with (
            tc.tile_pool(name="yT", bufs=2, space="PSUM") as ypool,
            tc.tile_pool(name="ob", bufs=4) as obpool,
        ):
            for h in range(NH):
                s = h % 2
                emitted = 0
                k = 0
                for qt in range(4):
                    yq = ypool.tile([128, 512], FP, name="yT")
                    for jj in range(4 * qt + 4):
                        if h + 1 < NH and emitted < LT and emitted <= k * 16 // 40:
                            score_tile(scA, scB, h + 1, (h + 1) % 2,
                                       JORDER[emitted])
                            emitted += 1
                        pv_quarter_mm(yq, h, s, qt, jj)
                        k += 1
                    quarter_tail(yq, h, qt)
                    if h == NH - 1:
                        for lt in range(4 * qt, 4 * qt + 4):
                            for n2 in range(2):
                                outproj_chunk(scA, scB, obpool, lt, n2)
                while h + 1 < NH and emitted < LT:
                    score_tile(scA, scB, h + 1, (h + 1) % 2, JORDER[emitted])
                    emitted += 1

    if DEBUG:
        dbg_yt = nc.declare_dram_parameter("dbg_yt", [256, L], BF,
                                           isOutput=True)
        dbg_vp = nc.declare_dram_parameter("dbg_vp", [256, LT * VW], BF,
                                           isOutput=True)
        dbg_pt = nc.declare_dram_parameter("dbg_pt", [128, L], BF,
                                           isOutput=True)
        for p in range(2):
            nc.sync.dma_start(out=dbg_yt[128 * p:128 * (p + 1), :],
                              in_=ytsb[p])
            nc.sync.dma_start(out=dbg_vp[128 * p:128 * (p + 1), :],
                              in_=vp[p])
        nc.sync.dma_start(out=dbg_pt[:, 0:L], in_=pts[0][0])
    perm.release()


_NC = None


def build_nc():
    global _NC
    if _NC is None:
        nc = bacc.Bacc("TRN2", target_bir_lowering=False)
        with tile.TileContext(nc) as tc:
            _emit(nc, tc)
        nc.finalize()
        _NC = nc
    return _NC


def prep_in_maps(x, g, W_qkv, W_out):
    x = np.asarray(x, dtype=np.float32)
    g = np.asarray(g, dtype=np.float32)
    W_qkv = np.asarray(W_qkv, dtype=np.float32)
    W_out = np.asarray(W_out, dtype=np.float32)
    bf = ml_dtypes.bfloat16
    xT = [np.ascontiguousarray(x[b].T).astype(bf) for b in range(B)]
    in_maps = []
    for c in range(NCORES):
        b, hg = c // 4, c % 4
        lo = PC * hg
        gTb = np.ascontiguousarray(g[b][:, lo:lo + PC].T).astype(bf)
        gzb = np.zeros((NH * 128, L), dtype=bf)
        for h in range(NH):
            r = 64 * (h % 2)
            gzb[128 * h + r:128 * h + r + 64, :] = gTb[64 * h:64 * h + 64, :]
        in_maps.append({
            "xT": xT[b],
            "gT": gTb,
            "gz": gzb,
            "wv": np.ascontiguousarray(
                W_qkv[:, 2 * D + lo:2 * D + lo + PC]).astype(bf),
            "wo": np.ascontiguousarray(W_out[lo:lo + PC, :]).astype(bf),
        })
    return in_maps


def gather(results):
    out = np.zeros((B, L, D), dtype=np.float32)
    for c in range(NCORES):
        out[c // 4] += np.asarray(results[c]["out_p"], dtype=np.float32)
    return out


def kernel(x, g, W_qkv, W_out):
    nc = build_nc()
    in_maps = prep_in_maps(x, g, W_qkv, W_out)
    res = run_bass_kernel_spmd(nc, in_maps, list(range(NCORES)))
    return gather(res.results)


# revision 20
# speedup vs baseline: 1.0927x; 1.0015x over previous
"""PathCausalSelfAttention on 8 trn2 cores.

Sharding: core c -> batch b=c//4, head-group hg=c%4 (4 heads each).

Key simplification vs the reference: the x-path score term is weighted
1e-6 and contributes ~1e-6 relative to the g-path, far below the 2e-2
tolerance, so q/k projections are dropped entirely. Scores are g.g per
head, exp on ACT, PV with a fused ones column for the softmax
denominator, out-projection in bf16.

v2 structure: ScalarE (exp over the causal triangle, ~57us of columns
at 0.82ns/col) pays ~293ns fixed cost per ACTIVATE, so score tiles are
built in two wide PSUM regions (A: 4 banks/2048 cols for key tiles
j<8, B: 2 banks/1024 cols for j>=8, emitted interleaved j,j+8 so the
regions ping-pong against the single act per (h,j) -- 64 acts total vs
96 before). Normalization: reciprocal on DVE straight from the PV
denominator row, gpsimd partition_broadcast of the fp32 row, then one
fused scalar_tensor_tensor multiply-evict into ytsb. The out-projection
rides the last head's quarters, borrowing the then-idle A/B PSUM banks.
Causal diag masks on GpSimd (SBUF only). Host sums 4 bf16 head-group
partials per batch.
"""

import numpy as np
import ml_dtypes

import concourse.bacc as bacc
import concourse.mybir as mybir
import concourse.tile as tile
from concourse import masks
from concourse.bass_utils import run_bass_kernel_spmd

B, L, D, H = 2, 2048, 1024, 16
HD = 64
NCORES = 8
NH = 4          # heads per core
PC = NH * HD    # 256 v / out-proj rows per core
FP = mybir.dt.float32
BF = mybir.dt.bfloat16
F8 = mybir.dt.float8e4
DR = mybir.MatmulPerfMode.DoubleRow
AF = mybir.ActivationFunctionType
ALU = mybir.AluOpType

LT = L // 128   # 16 L-tiles
DC = D // 128   # 8 contraction chunks
VW = 256  # V' cols per L-tile per pr: 2 heads x [ones | 63 pad | 64 v dims]

# j emission order: big tile (region A) alternating with small (region B)
JORDER = [0, 8, 1, 9, 2, 10, 3, 11, 4, 12, 5, 13, 6, 14, 7, 15]

ACT_W = 1024    # max exp width per ACTIVATE

DEBUG = False


def _emit(nc, tc):
    xT_d = nc.declare_dram_parameter("xT", [D, L], BF, isOutput=False)
    # per-head zero-padded key tiles: head h's 64 dims live in partition
    # rows 64*(h%2)..+64, other rows zero. Used as the score lhsT so the
    # contraction is full 128 rows and lhsT/rhs come from different tiles.
    gT_d = nc.declare_dram_parameter("gT", [PC, L], BF, isOutput=False)
    gz_d = nc.declare_dram_parameter("gz", [NH * 128, L], BF, isOutput=False)
    wv_d = nc.declare_dram_parameter("wv", [D, PC], BF, isOutput=False)
    wo_d = nc.declare_dram_parameter("wo", [PC, D], BF, isOutput=False)
    out_p = nc.declare_dram_parameter("out_p", [L, D], BF, isOutput=True)

    perm = tc.alloc_tile_pool(name="perm", bufs=1)
    I128 = perm.tile([128, 128], BF, name="I128")
    An = perm.tile([128, 128], BF, name="An")
    gt = [perm.tile([128, L], BF, name=f"gt{p}") for p in range(2)]
    gz = [perm.tile([128, L], BF, name=f"gz{h}") for h in range(NH)]
    xT = [perm.tile([128, L], BF, name=f"xT{d}") for d in range(DC)]
    wv = [perm.tile([128, PC], BF, name=f"wv{d}") for d in range(DC)]
    wo = [perm.tile([128, D], BF, name=f"wo{p}") for p in range(2)]
    vp = [perm.tile([128, LT * VW], BF, name=f"vp{p}") for p in range(2)]
    ytsb = [perm.tile([128, L], BF, name=f"yt{p}") for p in range(2)]
    # two ping-pong sets of per-head p tiles, trimmed to the causal width
    pts = [[perm.tile([128, L - 128 * j], BF, name=f"pt{s}_{j}")
            for j in range(LT)] for s in range(2)]
    rcp = [perm.tile([HD, 512], FP, name=f"rcp{i}") for i in range(2)]
    perm.seal()

    # DMAs: first-needed first. gz0/gt0 column-chunked so the first score
    # matmuls start as soon as the leading columns land.
    # DMAs: first-needed first. gz0/gt0 column-chunked so the first score
    # matmuls start as soon as the leading columns land.
    bounds = [0, 256, 512, 1024, 1536, 2048]
    for c4 in range(len(bounds) - 1):
        sl = slice(bounds[c4], bounds[c4 + 1])
        nc.sync.dma_start(out=gz[0][:, sl], in_=gz_d[0:128, sl])
        nc.sync.dma_start(out=gt[0][:, sl], in_=gT_d[0:128, sl])
    # column-halved xT so the early v-projection tiles (which read only
    # 128 cols of each d-chunk) unblock after half the bytes
    for d in range(DC):
        nc.sync.dma_start(out=xT[d][:, 0:1024],
                          in_=xT_d[128 * d:128 * (d + 1), 0:1024])
    for d in range(DC):
        nc.sync.dma_start(out=wv[d], in_=wv_d[128 * d:128 * (d + 1), :])
    for d in range(DC):
        nc.sync.dma_start(out=xT[d][:, 1024:2048],
                          in_=xT_d[128 * d:128 * (d + 1), 1024:2048])
    nc.sync.dma_start(out=gz[1], in_=gz_d[128:256, :])
    nc.sync.dma_start(out=gt[1], in_=gT_d[128:256, :])
    nc.sync.dma_start(out=gz[2], in_=gz_d[256:384, :])
    nc.sync.dma_start(out=gz[3], in_=gz_d[384:512, :])
    for p in range(2):
        nc.sync.dma_start(out=wo[p], in_=wo_d[128 * p:128 * (p + 1), :])

    masks.make_identity(nc, I128)
    masks.make_lower_triangular(nc, An, val=-240.0, diag=False)
    for p in range(2):
        # each 128-col head group is [64 ones | 64 v dims]: the PV matmul
        # then lands the softmax denominator on yq partitions 0..63
        # (replicated) and y on 64..127, so 1/den runs on DVE with no
        # cross-partition broadcast at all
        nc.vector.memset(vp[p], 1.0)

    def score_tile(scA, scB, h, s, j):
        """Score chain for (h, key tile j): matmuls into one wide PSUM
        region with the causal mask of the diagonal block folded in as an
        additive -240 matmul (I128.T @ An), then ONE exp act."""
        pr = h // 2
        W = L - 128 * j
        pool, cap = (scA, 2048) if j < 8 else (scB, 1024)
        ptj = pts[s][j]
        rt = pool.tile([128, cap], FP, name="rt")
        nc.tensor.matmul(rt[:, 0:128], lhsT=I128, rhs=An,
                         start=True, stop=False)
        nc.tensor.matmul(
            rt[:, 0:128],
            lhsT=gz[h][:, 128 * j:128 * j + 128],
            rhs=gt[pr][:, 128 * j:128 * j + 128],
            start=False, stop=True)
        c = 128
        while c < W:
            bw = min(512 - (c & 511), W - c)
            q0 = 128 * j + c
            nc.tensor.matmul(
                rt[:, c:c + bw],
                lhsT=gz[h][:, 128 * j:128 * j + 128],
                rhs=gt[pr][:, q0:q0 + bw],
                start=True, stop=True)
            c += bw
        nc.scalar.activation(ptj[:, 0:W], rt[:, 0:W], AF.Exp, scale=0.125)

    def vproj_tile(vpool, i):
        ps = vpool.tile([128, PC], FP, name="vps", padded_shape=[128, 512])
        for d in range(DC):
            nc.tensor.matmul(
                ps, lhsT=xT[d][:, 128 * i:128 * (i + 1)],
                rhs=wv[d],
                start=(d == 0), stop=(d == DC - 1))
        for pr in range(2):
            # both heads' 64-col v blocks in one strided cast per pr; cols
            # 0..63 of each group stay 1.0 (replicated softmax denominator)
            nc.vector.tensor_copy(
                vp[pr][:, VW * i:VW * i + VW].rearrange(
                    "p (hh c) -> p hh c", hh=2)[:, :, 64:64 + HD],
                ps[:, 128 * pr:128 * (pr + 1)].rearrange(
                    "p (hh c) -> p hh c", hh=2))

    def pv_quarter_mm(yq, h, s, qt, jj):
        pr, hh = h // 2, h % 2
        q0 = max(128 * jj, 512 * qt)
        q1 = 512 * (qt + 1)
        nc.tensor.matmul(
            yq[:, q0 - 512 * qt:q1 - 512 * qt],
            lhsT=vp[pr][:, VW * jj + 128 * hh:VW * jj + 128 * hh + 128],
            rhs=pts[s][jj][:, q0 - 128 * jj:q1 - 128 * jj],
            start=(jj == 0), stop=(jj == 4 * qt + 3))

    def quarter_tail(yq, h, qt):
        """After PV quarter qt stops: 1/den on DVE over the 64 replicated
        den rows, then one fused multiply-evict into ytsb."""
        pr, hh = h // 2, h % 2
        o = 512 * qt
        i = (4 * h + qt) % 2
        nc.vector.reciprocal_approx_fast(rcp[i][0:HD, 0:512],
                                         yq[0:HD, :])
        nc.vector.scalar_tensor_tensor(
            out=ytsb[pr][64 * hh:64 * hh + 64, o:o + 512],
            in0=yq[64:128, :], scalar=0.0, in1=rcp[i][0:HD, 0:512],
            op0=ALU.bypass, op1=ALU.mult)

    def outproj_chunk(scA, scB, obpool, lt, n2):
        # reuse the score-region slots (same tag) -- the A/B banks are idle
        # during the last head's quarters
        if (lt + n2) % 2 == 0:
            op = scA.tile([128, 512], FP, name="rt",
                          padded_shape=[128, 2048])
        else:
            op = scB.tile([128, 512], FP, name="rt",
                          padded_shape=[128, 1024])
        for pr in range(2):
            nc.tensor.matmul(
                op, lhsT=ytsb[pr][:, 128 * lt:128 * (lt + 1)],
                rhs=wo[pr][:, 512 * n2:512 * (n2 + 1)],
                start=(pr == 0), stop=(pr == 1))
        ob = obpool.tile([128, 512], BF, name="ob")
        # alternate evict engines: ScalarE is idle during the last head
        if (lt + n2) % 2 == 0:
            nc.vector.tensor_copy(ob, op)
        else:
            nc.scalar.copy(ob, op)
        nc.sync.dma_start(
            out=out_p[128 * lt:128 * (lt + 1), 512 * n2:512 * (n2 + 1)],
            in_=ob)

    with (
        tc.tile_pool(name="scA", bufs=1, space="PSUM") as scA,
        tc.tile_pool(name="scB", bufs=1, space="PSUM") as scB,
    ):
        # phase A/B: scores for head 0 with the v-projection interleaved
        with tc.tile_pool(name="vps", bufs=2, space="PSUM") as vpool:
            for idx, j in enumerate(JORDER):
                score_tile(scA, scB, 0, 0, j)
                if idx >= 6:
                    vproj_tile(vpool, idx - 6)
            for i in range(LT - 6, LT):
                vproj_tile(vpool, i)
            # head 1's first chains ride the tail of phase A: the region
            # ping-pong is free once head 0's last acts drain, and it pulls
            # all later score pacing earlier
            for e in range(4):
                score_tile(scA, scB, 1, 1, JORDER[e])

        # slots: PV(h) quarter-major, scores of head h+1 spread through;
        # the out-projection rides slot 3 per normalized query quarter.
        with (
            tc.tile_pool(name="yT", bufs=2, space="PSUM") as ypool,
            tc.tile_pool(name="ob", bufs=4) as obpool,
        ):
            for h in range(NH):
                s = h % 2
                emitted = 4 if h == 0 else 0
                k = 0
                for qt in range(4):
                    yq = ypool.tile([128, 512], FP, name="yT")
                    for jj in range(4 * qt + 4):
                        if h + 1 < NH and emitted < LT and emitted <= k * 16 // 40:
                            score_tile(scA, scB, h + 1, (h + 1) % 2,
                                       JORDER[emitted])
                            emitted += 1
                        pv_quarter_mm(yq, h, s, qt, jj)
                        k += 1
                    quarter_tail(yq, h, qt)
                    if h == NH - 1:
                        for lt in range(4 * qt, 4 * qt + 4):
                            for n2 in range(2):
                                outproj_chunk(scA, scB, obpool, lt, n2)
                while h + 1 < NH and emitted < LT:
                    score_tile(scA, scB, h + 1, (h + 1) % 2, JORDER[emitted])
                    emitted += 1

    if DEBUG:
        dbg_yt = nc.declare_dram_parameter("dbg_yt", [256, L], BF,
                                           isOutput=True)
        dbg_vp = nc.declare_dram_parameter("dbg_vp", [256, LT * VW], BF,
                                           isOutput=True)
        dbg_pt = nc.declare_dram_parameter("dbg_pt", [128, L], BF,
                                           isOutput=True)
        for p in range(2):
            nc.sync.dma_start(out=dbg_yt[128 * p:128 * (p + 1), :],
                              in_=ytsb[p])
            nc.sync.dma_start(out=dbg_vp[128 * p:128 * (p + 1), :],
                              in_=vp[p])
        nc.sync.dma_start(out=dbg_pt[:, 0:L], in_=pts[0][0])
    perm.release()


_NC = None


def build_nc():
    global _NC
    if _NC is None:
        nc = bacc.Bacc("TRN2", target_bir_lowering=False)
        with tile.TileContext(nc) as tc:
            _emit(nc, tc)
        nc.finalize()
        _NC = nc
    return _NC


def prep_in_maps(x, g, W_qkv, W_out):
    x = np.asarray(x, dtype=np.float32)
    g = np.asarray(g, dtype=np.float32)
    W_qkv = np.asarray(W_qkv, dtype=np.float32)
    W_out = np.asarray(W_out, dtype=np.float32)
    bf = ml_dtypes.bfloat16
    xT = [np.ascontiguousarray(x[b].T).astype(bf) for b in range(B)]
    in_maps = []
    for c in range(NCORES):
        b, hg = c // 4, c % 4
        lo = PC * hg
        gTb = np.ascontiguousarray(g[b][:, lo:lo + PC].T).astype(bf)
        gzb = np.zeros((NH * 128, L), dtype=bf)
        for h in range(NH):
            r = 64 * (h % 2)
            gzb[128 * h + r:128 * h + r + 64, :] = gTb[64 * h:64 * h + 64, :]
        in_maps.append({
            "xT": xT[b],
            "gT": gTb,
            "gz": gzb,
            "wv": np.ascontiguousarray(
                W_qkv[:, 2 * D + lo:2 * D + lo + PC]).astype(bf),
            "wo": np.ascontiguousarray(W_out[lo:lo + PC, :]).astype(bf),
        })
    return in_maps


def gather(results):
    out = np.zeros((B, L, D), dtype=np.float32)
    for c in range(NCORES):
        out[c // 4] += np.asarray(results[c]["out_p"], dtype=np.float32)
    return out


def kernel(x, g, W_qkv, W_out):
    nc = build_nc()
    in_maps = prep_in_maps(x, g, W_qkv, W_out)
    res = run_bass_kernel_spmd(nc, in_maps, list(range(NCORES)))
    return gather(res.results)


# revision 21
# speedup vs baseline: 1.0955x; 1.0025x over previous
"""PathCausalSelfAttention on 8 trn2 cores.

Sharding: core c -> batch b=c//4, head-group hg=c%4 (4 heads each).

Key simplification vs the reference: the x-path score term is weighted
1e-6 and contributes ~1e-6 relative to the g-path, far below the 2e-2
tolerance, so q/k projections are dropped entirely. Scores are g.g per
head, exp on ACT, PV with a fused ones column for the softmax
denominator, out-projection in bf16.

v2 structure: ScalarE (exp over the causal triangle, ~57us of columns
at 0.82ns/col) pays ~293ns fixed cost per ACTIVATE, so score tiles are
built in two wide PSUM regions (A: 4 banks/2048 cols for key tiles
j<8, B: 2 banks/1024 cols for j>=8, emitted interleaved j,j+8 so the
regions ping-pong against the single act per (h,j) -- 64 acts total vs
96 before). Normalization: reciprocal on DVE straight from the PV
denominator row, gpsimd partition_broadcast of the fp32 row, then one
fused scalar_tensor_tensor multiply-evict into ytsb. The out-projection
rides the last head's quarters, borrowing the then-idle A/B PSUM banks.
Causal diag masks on GpSimd (SBUF only). Host sums 4 bf16 head-group
partials per batch.
"""

import numpy as np
import ml_dtypes

import concourse.bacc as bacc
import concourse.mybir as mybir
import concourse.tile as tile
from concourse import masks
from concourse.bass_utils import run_bass_kernel_spmd

B, L, D, H = 2, 2048, 1024, 16
HD = 64
NCORES = 8
NH = 4          # heads per core
PC = NH * HD    # 256 v / out-proj rows per core
FP = mybir.dt.float32
BF = mybir.dt.bfloat16
F8 = mybir.dt.float8e4
DR = mybir.MatmulPerfMode.DoubleRow
AF = mybir.ActivationFunctionType
ALU = mybir.AluOpType

LT = L // 128   # 16 L-tiles
DC = D // 128   # 8 contraction chunks
VW = 256  # V' cols per L-tile per pr: 2 heads x [ones | 63 pad | 64 v dims]

# j emission order: big tile (region A) alternating with small (region B)
JORDER = [0, 8, 1, 9, 2, 10, 3, 11, 4, 12, 5, 13, 6, 14, 7, 15]

ACT_W = 1024    # max exp width per ACTIVATE

DEBUG = False


def _emit(nc, tc):
    xT_d = nc.declare_dram_parameter("xT", [D, L], BF, isOutput=False)
    # per-head zero-padded key tiles: head h's 64 dims live in partition
    # rows 64*(h%2)..+64, other rows zero. Used as the score lhsT so the
    # contraction is full 128 rows and lhsT/rhs come from different tiles.
    gT_d = nc.declare_dram_parameter("gT", [PC, L], BF, isOutput=False)
    gz_d = nc.declare_dram_parameter("gz", [NH * 128, L], BF, isOutput=False)
    wv_d = nc.declare_dram_parameter("wv", [D, PC], BF, isOutput=False)
    wo_d = nc.declare_dram_parameter("wo", [PC, D], BF, isOutput=False)
    out_p = nc.declare_dram_parameter("out_p", [L, D], BF, isOutput=True)

    perm = tc.alloc_tile_pool(name="perm", bufs=1)
    I128 = perm.tile([128, 128], BF, name="I128")
    An = perm.tile([128, 128], BF, name="An")
    gt = [perm.tile([128, L], BF, name=f"gt{p}") for p in range(2)]
    gz = [perm.tile([128, L], BF, name=f"gz{h}") for h in range(NH)]
    xT = [perm.tile([128, L], BF, name=f"xT{d}") for d in range(DC)]
    wv = [perm.tile([128, PC], BF, name=f"wv{d}") for d in range(DC)]
    wo = [perm.tile([128, D], BF, name=f"wo{p}") for p in range(2)]
    vp = [perm.tile([128, LT * VW], BF, name=f"vp{p}") for p in range(2)]
    ytsb = [perm.tile([128, L], BF, name=f"yt{p}") for p in range(2)]
    # two ping-pong sets of per-head p tiles, trimmed to the causal width
    pts = [[perm.tile([128, L - 128 * j], BF, name=f"pt{s}_{j}")
            for j in range(LT)] for s in range(2)]
    rcp = [perm.tile([HD, 512], FP, name=f"rcp{i}") for i in range(2)]
    perm.seal()

    # DMAs: first-needed first. gz0/gt0 column-chunked so the first score
    # matmuls start as soon as the leading columns land.
    # DMAs: first-needed first. gz0/gt0 column-chunked so the first score
    # matmuls start as soon as the leading columns land.
    bounds = [0, 256, 512, 1024, 1536, 2048]
    for c4 in range(len(bounds) - 1):
        sl = slice(bounds[c4], bounds[c4 + 1])
        nc.sync.dma_start(out=gz[0][:, sl], in_=gz_d[0:128, sl])
        nc.sync.dma_start(out=gt[0][:, sl], in_=gT_d[0:128, sl])
    # column-halved xT so the early v-projection tiles (which read only
    # 128 cols of each d-chunk) unblock after half the bytes
    for d in range(DC):
        nc.sync.dma_start(out=xT[d][:, 0:1024],
                          in_=xT_d[128 * d:128 * (d + 1), 0:1024])
    for d in range(DC):
        nc.sync.dma_start(out=wv[d], in_=wv_d[128 * d:128 * (d + 1), :])
    for d in range(DC):
        nc.sync.dma_start(out=xT[d][:, 1024:2048],
                          in_=xT_d[128 * d:128 * (d + 1), 1024:2048])
    nc.sync.dma_start(out=gz[1], in_=gz_d[128:256, :])
    nc.sync.dma_start(out=gt[1], in_=gT_d[128:256, :])
    nc.sync.dma_start(out=gz[2], in_=gz_d[256:384, :])
    nc.sync.dma_start(out=gz[3], in_=gz_d[384:512, :])
    for p in range(2):
        nc.sync.dma_start(out=wo[p], in_=wo_d[128 * p:128 * (p + 1), :])

    masks.make_identity(nc, I128)
    masks.make_lower_triangular(nc, An, val=-240.0, diag=False)
    for p in range(2):
        # each 128-col head group is [64 ones | 64 v dims]: the PV matmul
        # then lands the softmax denominator on yq partitions 0..63
        # (replicated) and y on 64..127, so 1/den runs on DVE with no
        # cross-partition broadcast at all
        nc.vector.memset(vp[p], 1.0)

    def score_tile(scA, scB, h, s, j, split_act=False):
        """Score chain for (h, key tile j): matmuls into one wide PSUM
        region with the causal mask of the diagonal block folded in as an
        additive -240 matmul (I128.T @ An), then ONE exp act (split in two
        for the first DMA-paced tiles of head 0 so exp starts while the
        trailing columns are still in flight)."""
        pr = h // 2
        W = L - 128 * j
        pool, cap = (scA, 2048) if j < 8 else (scB, 1024)
        ptj = pts[s][j]
        rt = pool.tile([128, cap], FP, name="rt")
        nc.tensor.matmul(rt[:, 0:128], lhsT=I128, rhs=An,
                         start=True, stop=False)
        nc.tensor.matmul(
            rt[:, 0:128],
            lhsT=gz[h][:, 128 * j:128 * j + 128],
            rhs=gt[pr][:, 128 * j:128 * j + 128],
            start=False, stop=True)
        c = 128
        while c < W:
            bw = min(512 - (c & 511), W - c)
            q0 = 128 * j + c
            nc.tensor.matmul(
                rt[:, c:c + bw],
                lhsT=gz[h][:, 128 * j:128 * j + 128],
                rhs=gt[pr][:, q0:q0 + bw],
                start=True, stop=True)
            c += bw
        if split_act and W > 1024:
            nc.scalar.activation(ptj[:, 0:1024], rt[:, 0:1024],
                                 AF.Exp, scale=0.125)
            nc.scalar.activation(ptj[:, 1024:W], rt[:, 1024:W],
                                 AF.Exp, scale=0.125)
        else:
            nc.scalar.activation(ptj[:, 0:W], rt[:, 0:W], AF.Exp,
                                 scale=0.125)

    def vproj_tile(vpool, i):
        ps = vpool.tile([128, PC], FP, name="vps", padded_shape=[128, 512])
        for d in range(DC):
            nc.tensor.matmul(
                ps, lhsT=xT[d][:, 128 * i:128 * (i + 1)],
                rhs=wv[d],
                start=(d == 0), stop=(d == DC - 1))
        for pr in range(2):
            # both heads' 64-col v blocks in one strided cast per pr; cols
            # 0..63 of each group stay 1.0 (replicated softmax denominator)
            nc.vector.tensor_copy(
                vp[pr][:, VW * i:VW * i + VW].rearrange(
                    "p (hh c) -> p hh c", hh=2)[:, :, 64:64 + HD],
                ps[:, 128 * pr:128 * (pr + 1)].rearrange(
                    "p (hh c) -> p hh c", hh=2))

    def pv_quarter_mm(yq, h, s, qt, jj):
        pr, hh = h // 2, h % 2
        q0 = max(128 * jj, 512 * qt)
        q1 = 512 * (qt + 1)
        nc.tensor.matmul(
            yq[:, q0 - 512 * qt:q1 - 512 * qt],
            lhsT=vp[pr][:, VW * jj + 128 * hh:VW * jj + 128 * hh + 128],
            rhs=pts[s][jj][:, q0 - 128 * jj:q1 - 128 * jj],
            start=(jj == 0), stop=(jj == 4 * qt + 3))

    def quarter_tail(yq, h, qt):
        """After PV quarter qt stops: 1/den on DVE over the 64 replicated
        den rows, then one fused multiply-evict into ytsb."""
        pr, hh = h // 2, h % 2
        o = 512 * qt
        i = (4 * h + qt) % 2
        nc.vector.reciprocal_approx_fast(rcp[i][0:HD, 0:512],
                                         yq[0:HD, :])
        nc.vector.scalar_tensor_tensor(
            out=ytsb[pr][64 * hh:64 * hh + 64, o:o + 512],
            in0=yq[64:128, :], scalar=0.0, in1=rcp[i][0:HD, 0:512],
            op0=ALU.bypass, op1=ALU.mult)

    def outproj_chunk(scA, scB, obpool, lt, n2):
        # reuse the score-region slots (same tag) -- the A/B banks are idle
        # during the last head's quarters
        if (lt + n2) % 2 == 0:
            op = scA.tile([128, 512], FP, name="rt",
                          padded_shape=[128, 2048])
        else:
            op = scB.tile([128, 512], FP, name="rt",
                          padded_shape=[128, 1024])
        for pr in range(2):
            nc.tensor.matmul(
                op, lhsT=ytsb[pr][:, 128 * lt:128 * (lt + 1)],
                rhs=wo[pr][:, 512 * n2:512 * (n2 + 1)],
                start=(pr == 0), stop=(pr == 1))
        ob = obpool.tile([128, 512], BF, name="ob")
        # alternate evict engines: ScalarE is idle during the last head
        if (lt + n2) % 2 == 0:
            nc.vector.tensor_copy(ob, op)
        else:
            nc.scalar.copy(ob, op)
        nc.sync.dma_start(
            out=out_p[128 * lt:128 * (lt + 1), 512 * n2:512 * (n2 + 1)],
            in_=ob)

    with (
        tc.tile_pool(name="scA", bufs=1, space="PSUM") as scA,
        tc.tile_pool(name="scB", bufs=1, space="PSUM") as scB,
    ):
        # phase A/B: scores for head 0 with the v-projection interleaved
        with tc.tile_pool(name="vps", bufs=2, space="PSUM") as vpool:
            for idx, j in enumerate(JORDER):
                score_tile(scA, scB, 0, 0, j, split_act=(idx < 6))
                if idx >= 6:
                    vproj_tile(vpool, idx - 6)
            for i in range(LT - 6, LT):
                vproj_tile(vpool, i)
            # head 1's first chains ride the tail of phase A: the region
            # ping-pong is free once head 0's last acts drain, and it pulls
            # all later score pacing earlier
            for e in range(6):
                score_tile(scA, scB, 1, 1, JORDER[e])

        # slots: PV(h) quarter-major, scores of head h+1 spread through;
        # the out-projection rides slot 3 per normalized query quarter.
        with (
            tc.tile_pool(name="yT", bufs=2, space="PSUM") as ypool,
            tc.tile_pool(name="ob", bufs=4) as obpool,
        ):
            for h in range(NH):
                s = h % 2
                emitted = 6 if h == 0 else 0
                k = 0
                for qt in range(4):
                    yq = ypool.tile([128, 512], FP, name="yT")
                    for jj in range(4 * qt + 4):
                        if h + 1 < NH and emitted < LT and emitted <= k * 16 // 40:
                            score_tile(scA, scB, h + 1, (h + 1) % 2,
                                       JORDER[emitted])
                            emitted += 1
                        pv_quarter_mm(yq, h, s, qt, jj)
                        k += 1
                    quarter_tail(yq, h, qt)
                    if h == NH - 1:
                        for lt in range(4 * qt, 4 * qt + 4):
                            for n2 in range(2):
                                outproj_chunk(scA, scB, obpool, lt, n2)
                while h + 1 < NH and emitted < LT:
                    score_tile(scA, scB, h + 1, (h + 1) % 2, JORDER[emitted])
                    emitted += 1

    if DEBUG:
        dbg_yt = nc.declare_dram_parameter("dbg_yt", [256, L], BF,
                                           isOutput=True)
        dbg_vp = nc.declare_dram_parameter("dbg_vp", [256, LT * VW], BF,
                                           isOutput=True)
        dbg_pt = nc.declare_dram_parameter("dbg_pt", [128, L], BF,
                                           isOutput=True)
        for p in range(2):
            nc.sync.dma_start(out=dbg_yt[128 * p:128 * (p + 1), :],
                              in_=ytsb[p])
            nc.sync.dma_start(out=dbg_vp[128 * p:128 * (p + 1), :],
                              in_=vp[p])
        nc.sync.dma_start(out=dbg_pt[:, 0:L], in_=pts[0][0])
    perm.release()


_NC = None


def build_nc():
    global _NC
    if _NC is None:
        nc = bacc.Bacc("TRN2", target_bir_lowering=False)
        with tile.TileContext(nc) as tc:
            _emit(nc, tc)
        nc.finalize()
        _NC = nc
    return _NC


def prep_in_maps(x, g, W_qkv, W_out):
    x = np.asarray(x, dtype=np.float32)
    g = np.asarray(g, dtype=np.float32)
    W_qkv = np.asarray(W_qkv, dtype=np.float32)
    W_out = np.asarray(W_out, dtype=np.float32)
    bf = ml_dtypes.bfloat16
    xT = [np.ascontiguousarray(x[b].T).astype(bf) for b in range(B)]
    in_maps = []
    for c in range(NCORES):
        b, hg = c // 4, c % 4
        lo = PC * hg
        gTb = np.ascontiguousarray(g[b][:, lo:lo + PC].T).astype(bf)
        gzb = np.zeros((NH * 128, L), dtype=bf)
        for h in range(NH):
            r = 64 * (h % 2)
            gzb[128 * h + r:128 * h + r + 64, :] = gTb[64 * h:64 * h + 64, :]
        in_maps.append({
            "xT": xT[b],
            "gT": gTb,
            "gz": gzb,
            "wv": np.ascontiguousarray(
                W_qkv[:, 2 * D + lo:2 * D + lo + PC]).astype(bf),
            "wo": np.ascontiguousarray(W_out[lo:lo + PC, :]).astype(bf),
        })
    return in_maps


def gather(results):
    out = np.zeros((B, L, D), dtype=np.float32)
    for c in range(NCORES):
        out[c // 4] += np.asarray(results[c]["out_p"], dtype=np.float32)
    return out


def kernel(x, g, W_qkv, W_out):
    nc = build_nc()
    in_maps = prep_in_maps(x, g, W_qkv, W_out)
    res = run_bass_kernel_spmd(nc, in_maps, list(range(NCORES)))
    return gather(res.results)
